# revision 45
# baseline (speedup 1.0000x reference)
"""Trainium2 Bass kernel for nn_O3TensorProductWeighted.

Computes, for each sample e:
    h  = relu(relu(weight @ W0 + b0) @ W1 + b1)           # [64]
    w  = h @ W2 + b2                                      # [36864] (never materialized)
    out0 = PW0*(einsum(Wa,s1)*s2 + I3*einsum(Wd,dot12))
    out1 = PW1*I3*(einsum(Wb,s1) x v2 + einsum(Wc,v1)*s2)
    out  = concat(out0, out1)/SQRT_K ; out[:128] += bias

Strategy (flipped-orientation): every per-sample einsum is reassociated
against the joint (k,u) contraction of the Khatri-Rao product h (x) x.
Unlike the streamed-samples formulation, the matmuls here use the
per-sample KR block as the *stationary* operand ([128 (k,u) rows x 128
samples]) and stream the *W2 chunk* as the moving operand, so each pass
costs only the number of output features it feeds (192 for the fused A|B
paths, 64 for C, 128 for D) instead of the full sample count.  All four
e-tiles of one output group accumulate into a single shared PSUM bank
opened by a full-bank-width bias matmul (guaranteeing ordering via WAW
deps).  The h-pair broadcast is done by a DMA with a 0-stride partition
AP (no PE/ACT cost), and the KR products are built by DVE (4 blocks) and
Pool (2 blocks) in parallel.  Pure data parallel over 8 cores.
"""

import dataclasses
import sys

sys.path.insert(0, "/opt/trn_rl_repo")

from contextlib import ExitStack

import ml_dtypes
import numpy as np

import concourse.bacc as bacc
import concourse.bass as bass
import concourse.tile as tile
from concourse import mybir
from concourse.bass_utils import run_bass_kernel_spmd

BF16 = mybir.dt.bfloat16
F32 = mybir.dt.float32
BF16_NP = ml_dtypes.bfloat16

N_CORES = 8
N = 4096
E = N // N_CORES  # 512 samples per core
ET = 4            # e-tiles of 128 samples

MUL0, MUL1 = 128, 64
N1 = MUL0 * MUL0          # 16384
N2 = MUL0 * MUL1          # 8192
N3 = MUL1 * MUL1          # 4096
I3 = float(1.0 / np.sqrt(3.0))
# PW0/SQRT_K == 1.0 and PW1*I3/SQRT_K == 1.0 exactly; only I3 remains on D,
# folded into the mvd weights host-side.

G = 32  # chunks; chunk g covers k in {2g, 2g+1}

# xblk block order: 0=AB-alpha (s1t), 1=AB-beta ([s1hi;s1lo]), 2=D, 3=C0,
# 4=C1, 5=C2.  DVE builds blocks 0:3 plus the first e-tile of block 3;
# Pool (which must stay off PSUM) builds the rest.
NBLK = 6
NBC = 4    # first NBC chunks build the h-pair broadcast on the (idle) PE
MLPC = 646 + NBC * 128  # mlppack column count


def _build_nc():
    nc = bacc.Bacc(None)

    # per-core inputs
    mlppack_d = nc.declare_dram_parameter("mlppack", [128, MLPC], BF16, isOutput=False)
    xblk_d = nc.declare_dram_parameter("xblk", [128, NBLK * E], BF16, isOutput=False)
    sv_d = nc.declare_dram_parameter("sv", [128, 16], F32, isOutput=False)
    dinit_d = nc.declare_dram_parameter("dinit", [65, E], BF16, isOutput=False)

    # replicated parameters
    mvab_d = nc.declare_dram_parameter("mvab", [128, G * 2 * 192], BF16, isOutput=False)
    mvc_d = nc.declare_dram_parameter("mvc", [128, G * 64], BF16, isOutput=False)
    mvd_d = nc.declare_dram_parameter("mvd", [128, G * 128], BF16, isOutput=False)
    b2abz_d = nc.declare_dram_parameter("b2abz", [128, 512], BF16, isOutput=False)
    b2c_d = nc.declare_dram_parameter("b2c", [64, 64], BF16, isOutput=False)
    b2dx_d = nc.declare_dram_parameter("b2dx", [65, 128], BF16, isOutput=False)

    outp_d = nc.declare_dram_parameter("outp", [E, 320], F32, isOutput=True)

    with tile.TileContext(nc) as tc, ExitStack() as ctx:
        const = ctx.enter_context(tc.tile_pool(name="const", bufs=1))
        work = ctx.enter_context(tc.tile_pool(name="work", bufs=1))
        bct_pool = ctx.enter_context(tc.tile_pool(name="bct", bufs=4))
        pt_pool = ctx.enter_context(tc.tile_pool(name="pt", bufs=6))
        ps_mlp = ctx.enter_context(tc.tile_pool(name="ps_mlp", bufs=2, space="PSUM"))
        ps_acc = ctx.enter_context(tc.tile_pool(name="ps_acc", bufs=1, space="PSUM"))

        def load(dparam, engine, pool=const):
            t = pool.tile(dparam.shape, dparam.dtype, name=f"t_{dparam.name}")
            engine.dma_start(t[:], dparam[:])
            return t

        # SP: mlp pack first (critical path), then xblk blocks 0:4 (DVE's
        # operands), then the mv chunks in g order so chunk g's weights
        # arrive just ahead of use; sv (epilogue-only) last.
        mlppack_t = load(mlppack_d, nc.sync)
        xblk_t = const.tile([128, NBLK * E], BF16)
        xblk3 = xblk_t[:].rearrange("p (b e) -> p b e", b=NBLK)
        xblk_d3 = xblk_d[:].rearrange("p (b e) -> p b e", b=NBLK)
        nc.sync.dma_start(xblk3[:, 0, :], xblk_d3[:, 0, :])
        nc.sync.dma_start(xblk3[:, 1:4, :], xblk_d3[:, 1:4, :])
        mvab_t = const.tile([128, G * 2 * 192], BF16)
        mvc_t = const.tile([128, G * 64], BF16)
        mvd_t = const.tile([128, G * 128], BF16)
        mvab3 = mvab_t[:].rearrange("p (g s c) -> p (g s) c", g=G, s=2)
        mvc3 = mvc_t[:].rearrange("p (g c) -> p g c", g=G)
        mvd3 = mvd_t[:].rearrange("p (g c) -> p g c", g=G)
        mvab_d3 = mvab_d[:].rearrange("p (g s c) -> p (g s) c", g=G, s=2)
        mvc_d3 = mvc_d[:].rearrange("p (g c) -> p g c", g=G)
        mvd_d3 = mvd_d[:].rearrange("p (g c) -> p g c", g=G)
        # batched in blocks of 4 chunks: per-transfer bytes stay above the
        # 500ns descriptor floor while chunk 0 is ready early
        GB = 4
        mvab_b = mvab_t[:].rearrange("p (b c) -> p b c", b=G // GB)
        mvc_b = mvc_t[:].rearrange("p (b c) -> p b c", b=G // GB)
        mvd_b = mvd_t[:].rearrange("p (b c) -> p b c", b=G // GB)
        mvab_db = mvab_d[:].rearrange("p (b c) -> p b c", b=G // GB)
        mvc_db = mvc_d[:].rearrange("p (b c) -> p b c", b=G // GB)
        mvd_db = mvd_d[:].rearrange("p (b c) -> p b c", b=G // GB)
        for b in range(G // GB):
            nc.sync.dma_start(mvab_b[:, b, :], mvab_db[:, b, :])
            nc.sync.dma_start(mvc_b[:, b, :], mvc_db[:, b, :])
            nc.sync.dma_start(mvd_b[:, b, :], mvd_db[:, b, :])
        sv_t = load(sv_d, nc.sync)

        # Pool: xblk blocks 4:6 (its own operands), then bias-init tensors
        nc.gpsimd.dma_start(xblk3[:, 4:6, :], xblk_d3[:, 4:6, :])
        b2abz_t = load(b2abz_d, nc.gpsimd)
        dinit_t = load(dinit_d, nc.gpsimd)
        b2dx_t = load(b2dx_d, nc.gpsimd)
        b2c_t = load(b2c_d, nc.gpsimd)

        # warm the ACT function table (Relu) before the MLP needs it
        warm_t = work.tile([1, 1], F32)
        nc.scalar.activation(warm_t[:], nc.const_aps.tensor(0.0, (1, 1)),
                             mybir.ActivationFunctionType.Relu)

        # --- MLP: h1 = relu(W0.T wT + b0); h2 = relu(W1.T h1 + b1) ---
        wT = mlppack_t[0:16, 0:512]
        w0 = mlppack_t[0:16, 512:576]
        w1 = mlppack_t[0:64, 576:640]
        b0c = mlppack_t[0:64, 640:641]
        b1c = mlppack_t[0:64, 641:642]

        ps_h1 = ps_mlp.tile([64, E], F32, tag="mlp")
        nc.tensor.matmul(ps_h1[:], w0, wT, start=True, stop=True)
        h1_t = work.tile([64, E], BF16)
        nc.scalar.activation(h1_t[:], ps_h1[:],
                             mybir.ActivationFunctionType.Relu,
                             bias=b0c, scale=1.0)

        # --- PSUM accumulator banks: one 2KB bank per e-tile ---
        # cols 0:192 = A|B, 192:320 = D (incl. out-bias), 320+64i = C_i
        # Each bank is opened by one full-bank-width start=True matmul (the
        # A|B bias init padded with zeros); everything after accumulates
        # with start=False, ordered by WAW deps through the opener.
        banks = [ps_acc.tile([128, 512], F32, tag=f"bk{t}", name=f"bank{t}")
                 for t in range(ET)]

        # first NBC chunks: h-pair broadcast fused into a PE matmul against
        # W1 column-pairs repeated 64x (skips the h2 -> DMA latency chain)
        bct_pe = []
        for g in range(NBC):
            ps_bc = ps_mlp.tile([128, E], F32, tag="bc")
            nc.tensor.matmul(ps_bc[:], mlppack_t[0:64, 646 + 128 * g:
                                                  646 + 128 * (g + 1)],
                             h1_t[:], start=True, stop=True)
            bct = bct_pool.tile([128, E], BF16, tag="bct")
            nc.scalar.activation(bct[:], ps_bc[:],
                                 mybir.ActivationFunctionType.Relu,
                                 bias=mlppack_t[:, 642 + g:643 + g],
                                 scale=1.0)
            bct_pe.append(bct)

        ps_h2 = ps_mlp.tile([64, E], F32, tag="mlp")
        nc.tensor.matmul(ps_h2[:], w1, h1_t[:], start=True, stop=True)
        h2_t = work.tile([64, E], BF16)
        nc.scalar.activation(h2_t[:], ps_h2[:],
                             mybir.ActivationFunctionType.Relu,
                             bias=b1c, scale=1.0)

        def ets(t):
            return bass.ts(t, 128)

        # bank openers + b2 bias contributions.  The D path (and its bias
        # init, including the final out0 bias via the 1/s2 row) carries a
        # host-side 1/s2 per-sample factor and accumulates straight into
        # the A region, so out0 is just (A+D') * s2 at the end.
        for t in range(ET):
            nc.tensor.matmul(banks[t][:, 0:384], xblk3[:, 0, ets(t)],
                             b2abz_t[:, 0:384], start=True, stop=False,
                             skip_group_check=True)
            nc.tensor.matmul(banks[t][:, 0:128], dinit_t[:, ets(t)],
                             b2dx_t[:], start=False, stop=False,
                             skip_group_check=True)
            for ci, blk in enumerate((3, 4, 5)):
                nc.tensor.matmul(banks[t][:, 192 + 64 * ci:256 + 64 * ci],
                                 xblk3[0:64, blk, ets(t)], b2c_t[:],
                                 start=False, stop=False,
                                 skip_group_check=True)

        # --- main loop over 32 chunks ---
        for g in range(G):
            if g < NBC:
                bct = bct_pe[g]
            else:
                # h-pair broadcast via DMA (issued on ACT queue): rows 0:64
                # = h2[2g], rows 64:128 = h2[2g+1]
                bct = bct_pool.tile([128, E], BF16, tag="bct")
                h_src = h2_t[2 * g:2 * g + 2, :]
                h_bc = dataclasses.replace(
                    h_src, ap=[h_src.ap[0], [0, 64], [1, E]])
                nc.scalar.dma_start(bct[:], h_bc)

            # KR products: pt[:, j, :] = xblk[j] * bct
            # DVE: blocks 0:3 + first e-tile of block 3 (C0); Pool: rest.
            pt = pt_pool.tile([128, NBLK * E], BF16, tag="pt")
            pt3 = pt[:].rearrange("p (b e) -> p b e", b=NBLK)
            bct3b = dataclasses.replace(
                bct[:], ap=[bct[:].ap[0], [0, 3], [1, E]])
            bct2 = dataclasses.replace(
                bct[:], ap=[bct[:].ap[0], [0, 2], [1, E]])
            if g < 2:
                # finer grain at startup so PE can begin on block 0 early
                bct1 = dataclasses.replace(
                    bct[:], ap=[bct[:].ap[0], [0, 1], [1, E]])
                nc.vector.tensor_mul(pt3[:, 0:1, :], xblk3[:, 0:1, :], bct1)
                bct2b = dataclasses.replace(
                    bct[:], ap=[bct[:].ap[0], [0, 2], [1, E]])
                nc.vector.tensor_mul(pt3[:, 1:3, :], xblk3[:, 1:3, :], bct2b)
            else:
                nc.vector.tensor_mul(pt3[:, 0:3, :], xblk3[:, 0:3, :], bct3b)
            nc.vector.tensor_mul(pt3[:, 3, 0:256], xblk3[:, 3, 0:256],
                                 bct[:, 0:256])
            nc.gpsimd.tensor_mul(pt3[:, 3, 256:E], xblk3[:, 3, 256:E],
                                 bct[:, 256:E])
            nc.gpsimd.tensor_mul(pt3[:, 4:6, :], xblk3[:, 4:6, :], bct2)

            last = g == G - 1
            for t in range(ET):
                sl = ets(t)
                nc.tensor.matmul(banks[t][:, 0:192], pt3[:, 0, sl],
                                 mvab3[:, 2 * g, :], start=False, stop=False,
                                 skip_group_check=True)
                nc.tensor.matmul(banks[t][:, 0:192], pt3[:, 1, sl],
                                 mvab3[:, 2 * g + 1, :], start=False,
                                 stop=last, skip_group_check=True)
                nc.tensor.matmul(banks[t][:, 0:128], pt3[:, 2, sl],
                                 mvd3[:, g, :], start=False, stop=last,
                                 skip_group_check=True)
                for ci, blk in enumerate((3, 4, 5)):
                    nc.tensor.matmul(
                        banks[t][:, 192 + 64 * ci:256 + 64 * ci],
                        pt3[:, blk, sl], mvc3[:, g, :], start=False,
                        stop=last, skip_group_check=True)

        # --- epilogue: out0 = (psA + psD') * s2 on ACT (per-partition
        # scale); for out1, DVE evacuates the B|C psum region to SBUF and
        # Pool (fast, but barred from PSUM) finishes
        # out1_i = sbB*v2_i + sbC_i.
        outS = work.tile([128, ET * 320], F32)
        oS = outS[:].rearrange("p (t c) -> p t c", t=ET)
        bcS = work.tile([128, ET * 256], F32)
        bc3 = bcS[:].rearrange("p (t c) -> p t c", t=ET)
        tmp1 = work.tile([128, ET * 192], F32)
        tm3 = tmp1[:].rearrange("p (t c) -> p t c", t=ET)
        for t in range(ET):
            nc.scalar.mul(oS[:, t, 0:128], banks[t][:, 0:128],
                          sv_t[:, t:t + 1])
            # bank 2's evacuation rides on ACT so DVE reaches the last
            # bank's copy sooner
            ceng = nc.scalar if t == 2 else nc.vector
            if t == 2:
                nc.scalar.copy(bc3[:, t, :], banks[t][:, 128:384])
            else:
                nc.vector.tensor_copy(bc3[:, t, :], banks[t][:, 128:384])
            # Pool may only use plain TensorTensor: one broadcast multiply
            # covering all 3 vector components, then one add of the C part.
            sbB = bc3[:, t, 0:64]
            sbB_bc = dataclasses.replace(
                sbB, ap=[sbB.ap[0], [0, 3], [1, 64]])
            svv = sv_t[:, 4 + t:5 + t]
            svv_bc = dataclasses.replace(
                svv, ap=[svv.ap[0], [4, 3], [0, 64]])
            t3 = tm3[:, t, :].rearrange("p (i w) -> p i w", i=3)
            nc.gpsimd.tensor_mul(t3, sbB_bc, svv_bc)
            o1 = oS[:, t, 128:320].rearrange("p (w i) -> p i w", i=3)
            nc.gpsimd.tensor_add(o1, t3, bc3[:, t, 64:256].rearrange(
                "p (i w) -> p i w", i=3))
            if t == ET - 1:
                nc.scalar.dma_start(outp_d[ets(t), 0:160], oS[:, t, 0:160])
                nc.sync.dma_start(outp_d[ets(t), 160:320], oS[:, t, 160:320])
            else:
                eng = nc.scalar if t % 2 == 0 else nc.sync
                eng.dma_start(outp_d[ets(t), :], oS[:, t, :])

    nc.compile()
    return nc


_NC = None


def _get_nc():
    global _NC
    if _NC is None:
        _NC = _build_nc()
    return _NC


def _prep_inputs(data_in1, data_in2, weight, W0, b0, W1, b1, W2, b2, bias):
    f32 = np.float32
    data_in1 = np.ascontiguousarray(data_in1, dtype=f32)
    data_in2 = np.ascontiguousarray(data_in2, dtype=f32)
    weight = np.ascontiguousarray(weight, dtype=f32)
    W0 = np.asarray(W0, f32); b0 = np.asarray(b0, f32)
    W1 = np.asarray(W1, f32); b1 = np.asarray(b1, f32)
    W2 = np.asarray(W2, f32); b2 = np.asarray(b2, f32)
    bias = np.asarray(bias, f32)

    s1 = data_in1[:, :MUL0]                      # [N,128]
    v1 = data_in1[:, MUL0:].reshape(N, MUL1, 3)  # [N,64,3]
    s2 = data_in2[:, 0]                          # [N]
    v2 = data_in2[:, 1:4]                        # [N,3]

    def bf(x):
        return np.ascontiguousarray(x, dtype=f32).astype(BF16_NP)

    s1t = s1.T                                   # [128,N]
    vs = [(v1[:, :, i] * s2[:, None]).T for i in range(3)]   # [64,N] each
    dot12 = np.einsum("eui,ei->eu", v1, v2).T    # [64,N]

    # xblk blocks: 0=s1t, 1=[s1hi;s1lo], 2=[d2/s2;d2/s2], 3=[vs0;vs0],
    # 4=[vs1;vs1], 5=[vs2;vs2].  Block 2 carries 1/s2 so the D path can
    # share the A accumulator (out0 = (A + D') * s2 exactly).
    d2s = dot12 / s2[None, :]
    xblk_full = np.stack([
        s1t,
        np.concatenate([s1t[64:128], s1t[0:64]], axis=0),
        np.concatenate([d2s, d2s], axis=0),
        np.concatenate([vs[0], vs[0]], axis=0),
        np.concatenate([vs[1], vs[1]], axis=0),
        np.concatenate([vs[2], vs[2]], axis=0),
    ], axis=1)                                   # [128, 6, N] f32
    xblk_full = bf(xblk_full)

    # W2 instruction blocks
    Wa3 = W2[:, :N1].reshape(64, 128, 128)
    Wb3 = W2[:, N1:N1 + N2].reshape(64, 128, 64)
    Wc3 = W2[:, N1 + N2:N1 + N2 + N3].reshape(64, 64, 64)
    Wd3 = W2[:, N1 + N2 + N3:].reshape(64, 64, 128)

    # mvab[r, g, s, :]: r<64 -> k=2g, u = r (s=0) or 64+r (s=1)
    #                   r>=64 -> k=2g+1, u = r (s=0) or r-64 (s=1)
    ks = np.arange(G * 2).reshape(G, 2)          # k for (g, half)
    mvab = np.empty((128, G, 2, 192), dtype=f32)
    r_lo = np.arange(64)
    # s=0: (k=2g, u=r_lo) | (k=2g+1, u=64+r_lo)
    mvab[0:64, :, 0, 0:128] = Wa3[ks[:, 0]][:, r_lo].transpose(1, 0, 2)
    mvab[64:128, :, 0, 0:128] = Wa3[ks[:, 1]][:, 64 + r_lo].transpose(1, 0, 2)
    mvab[0:64, :, 0, 128:192] = Wb3[ks[:, 0]][:, r_lo].transpose(1, 0, 2)
    mvab[64:128, :, 0, 128:192] = Wb3[ks[:, 1]][:, 64 + r_lo].transpose(1, 0, 2)
    # s=1: (k=2g, u=64+r_lo) | (k=2g+1, u=r_lo)
    mvab[0:64, :, 1, 0:128] = Wa3[ks[:, 0]][:, 64 + r_lo].transpose(1, 0, 2)
    mvab[64:128, :, 1, 0:128] = Wa3[ks[:, 1]][:, r_lo].transpose(1, 0, 2)
    mvab[0:64, :, 1, 128:192] = Wb3[ks[:, 0]][:, 64 + r_lo].transpose(1, 0, 2)
    mvab[64:128, :, 1, 128:192] = Wb3[ks[:, 1]][:, r_lo].transpose(1, 0, 2)

    # mvc/mvd[r, g, :]: r<64 -> (k=2g, u=r); r>=64 -> (k=2g+1, u=r-64)
    mvc = np.empty((128, G, 64), dtype=f32)
    mvc[0:64] = Wc3[ks[:, 0]][:, r_lo].transpose(1, 0, 2)
    mvc[64:128] = Wc3[ks[:, 1]][:, r_lo].transpose(1, 0, 2)
    mvd = np.empty((128, G, 128), dtype=f32)
    mvd[0:64] = I3 * Wd3[ks[:, 0]][:, r_lo].transpose(1, 0, 2)
    mvd[64:128] = I3 * Wd3[ks[:, 1]][:, r_lo].transpose(1, 0, 2)

    b2abz = np.zeros((128, 512), dtype=f32)
    b2abz[:, 0:128] = b2[:N1].reshape(128, 128)
    b2abz[:, 128:192] = b2[N1:N1 + N2].reshape(128, 64)
    b2c = b2[N1 + N2:N1 + N2 + N3].reshape(64, 64)
    b2dx = np.empty((65, 128), dtype=f32)
    b2dx[0:64] = I3 * b2[N1 + N2 + N3:].reshape(64, 128)
    b2dx[64] = bias

    shared = {
        "mvab": bf(mvab.reshape(128, -1)),
        "mvc": bf(mvc.reshape(128, -1)),
        "mvd": bf(mvd.reshape(128, -1)),
        "b2abz": bf(b2abz),
        "b2c": bf(b2c),
        "b2dx": bf(b2dx),
    }

    in_maps = []
    for c in range(N_CORES):
        e0 = c * E
        m = dict(shared)
        mlppack = np.zeros((128, MLPC), dtype=f32)
        mlppack[0:16, 0:512] = weight[e0:e0 + E].T
        mlppack[0:16, 512:576] = W0
        mlppack[0:64, 576:640] = W1
        mlppack[0:64, 640] = b0
        mlppack[0:64, 641] = b1
        for g in range(NBC):
            mlppack[0:64, 642 + g] = b1[2 * g]
            mlppack[64:128, 642 + g] = b1[2 * g + 1]
            # W1 column-pair repeated 64x: col r of block g is
            # W1[:, 2g + (r >= 64)]
            mlppack[0:64, 646 + 128 * g:646 + 128 * g + 64] = \
                W1[:, 2 * g:2 * g + 1]
            mlppack[0:64, 646 + 128 * g + 64:646 + 128 * (g + 1)] = \
                W1[:, 2 * g + 1:2 * g + 2]
        m["mlppack"] = bf(mlppack)
        m["xblk"] = np.ascontiguousarray(
            xblk_full[:, :, e0:e0 + E]).reshape(128, NBLK * E)
        sv = np.empty((128, 16), dtype=f32)
        for t in range(ET):
            t0 = e0 + t * 128
            sv[:, t] = s2[t0:t0 + 128]
            for i in range(3):
                sv[:, 4 + 4 * i + t] = v2[t0:t0 + 128, i]
        m["sv"] = sv
        dinit = np.empty((65, E), dtype=f32)
        dinit[0:64] = d2s[:, e0:e0 + E]
        dinit[64] = 1.0 / s2[e0:e0 + E]
        m["dinit"] = bf(dinit)
        in_maps.append(m)
    return in_maps


def run(in_maps, **kwargs):
    nc = _get_nc()
    return run_bass_kernel_spmd(nc, in_maps, list(range(N_CORES)), **kwargs)


def kernel(data_in1, data_in2, weight, W0, b0, W1, b1, W2, b2, bias):
    in_maps = _prep_inputs(
        data_in1, data_in2, weight, W0, b0, W1, b1, W2, b2, bias
    )
    res = run(in_maps)
    out = np.concatenate(
        [np.asarray(res.results[c]["outp"]) for c in range(N_CORES)], axis=0
    )
    return out.astype(np.float32)


# revision 47
# speedup vs baseline: 1.0091x; 1.0091x over previous
"""Trainium2 Bass kernel for nn_O3TensorProductWeighted.

Computes, for each sample e:
    h  = relu(relu(weight @ W0 + b0) @ W1 + b1)           # [64]
    w  = h @ W2 + b2                                      # [36864] (never materialized)
    out0 = PW0*(einsum(Wa,s1)*s2 + I3*einsum(Wd,dot12))
    out1 = PW1*I3*(einsum(Wb,s1) x v2 + einsum(Wc,v1)*s2)
    out  = concat(out0, out1)/SQRT_K ; out[:128] += bias

Strategy (flipped-orientation): every per-sample einsum is reassociated
against the joint (k,u) contraction of the Khatri-Rao product h (x) x.
Unlike the streamed-samples formulation, the matmuls here use the
per-sample KR block as the *stationary* operand ([128 (k,u) rows x 128
samples]) and stream the *W2 chunk* as the moving operand, so each pass
costs only the number of output features it feeds (192 for the fused A|B
paths, 64 for C, 128 for D) instead of the full sample count.  All four
e-tiles of one output group accumulate into a single shared PSUM bank
opened by a full-bank-width bias matmul (guaranteeing ordering via WAW
deps).  The h-pair broadcast is done by a DMA with a 0-stride partition
AP (no PE/ACT cost), and the KR products are built by DVE (4 blocks) and
Pool (2 blocks) in parallel.  Pure data parallel over 8 cores.
"""

import dataclasses
import sys

sys.path.insert(0, "/opt/trn_rl_repo")

from contextlib import ExitStack

import ml_dtypes
import numpy as np

import concourse.bacc as bacc
import concourse.bass as bass
import concourse.tile as tile
from concourse import mybir
from concourse.bass_utils import run_bass_kernel_spmd

BF16 = mybir.dt.bfloat16
F32 = mybir.dt.float32
BF16_NP = ml_dtypes.bfloat16

N_CORES = 8
N = 4096
E = N // N_CORES  # 512 samples per core
ET = 4            # e-tiles of 128 samples

MUL0, MUL1 = 128, 64
N1 = MUL0 * MUL0          # 16384
N2 = MUL0 * MUL1          # 8192
N3 = MUL1 * MUL1          # 4096
I3 = float(1.0 / np.sqrt(3.0))
# PW0/SQRT_K == 1.0 and PW1*I3/SQRT_K == 1.0 exactly; only I3 remains on D,
# folded into the mvd weights host-side.

G = 32  # chunks; chunk g covers k in {2g, 2g+1}

# xblk block order: 0=AB-alpha (s1t), 1=AB-beta ([s1hi;s1lo]), 2=D, 3=C0,
# 4=C1, 5=C2.  DVE builds blocks 0:3 plus the first e-tile of block 3;
# Pool (which must stay off PSUM) builds the rest.
NBLK = 6
NBC = 4    # first NBC chunks build the h-pair broadcast on the (idle) PE
MLPC = 646 + NBC * 128  # mlppack column count


def _build_nc():
    nc = bacc.Bacc(None)

    # per-core inputs
    mlppack_d = nc.declare_dram_parameter("mlppack", [128, MLPC], BF16, isOutput=False)
    xblk_d = nc.declare_dram_parameter("xblk", [128, NBLK * E], BF16, isOutput=False)
    sv_d = nc.declare_dram_parameter("sv", [128, 16], F32, isOutput=False)
    dinit_d = nc.declare_dram_parameter("dinit", [65, E], BF16, isOutput=False)

    # replicated parameters
    mvab_d = nc.declare_dram_parameter("mvab", [128, G * 2 * 192], BF16, isOutput=False)
    mvc_d = nc.declare_dram_parameter("mvc", [128, G * 64], BF16, isOutput=False)
    mvd_d = nc.declare_dram_parameter("mvd", [128, G * 128], BF16, isOutput=False)
    b2abz_d = nc.declare_dram_parameter("b2abz", [128, 512], BF16, isOutput=False)
    b2c_d = nc.declare_dram_parameter("b2c", [64, 64], BF16, isOutput=False)
    b2dx_d = nc.declare_dram_parameter("b2dx", [65, 128], BF16, isOutput=False)

    outp_d = nc.declare_dram_parameter("outp", [E, 320], F32, isOutput=True)

    with tile.TileContext(nc) as tc, ExitStack() as ctx:
        const = ctx.enter_context(tc.tile_pool(name="const", bufs=1))
        work = ctx.enter_context(tc.tile_pool(name="work", bufs=1))
        bct_pool = ctx.enter_context(tc.tile_pool(name="bct", bufs=4))
        pt_pool = ctx.enter_context(tc.tile_pool(name="pt", bufs=6))
        ps_mlp = ctx.enter_context(tc.tile_pool(name="ps_mlp", bufs=2, space="PSUM"))
        ps_acc = ctx.enter_context(tc.tile_pool(name="ps_acc", bufs=1, space="PSUM"))

        def load(dparam, engine, pool=const):
            t = pool.tile(dparam.shape, dparam.dtype, name=f"t_{dparam.name}")
            engine.dma_start(t[:], dparam[:])
            return t

        # SP: mlp pack first (critical path), then xblk blocks 0:4 (DVE's
        # operands), then the mv chunks in g order so chunk g's weights
        # arrive just ahead of use; sv (epilogue-only) last.
        mlppack_t = load(mlppack_d, nc.sync)
        xblk_t = const.tile([128, NBLK * E], BF16)
        xblk3 = xblk_t[:].rearrange("p (b e) -> p b e", b=NBLK)
        xblk_d3 = xblk_d[:].rearrange("p (b e) -> p b e", b=NBLK)
        nc.sync.dma_start(xblk3[:, 0, :], xblk_d3[:, 0, :])
        nc.sync.dma_start(xblk3[:, 1:4, :], xblk_d3[:, 1:4, :])
        mvab_t = const.tile([128, G * 2 * 192], BF16)
        mvc_t = const.tile([128, G * 64], BF16)
        mvd_t = const.tile([128, G * 128], BF16)
        mvab3 = mvab_t[:].rearrange("p (g s c) -> p (g s) c", g=G, s=2)
        mvc3 = mvc_t[:].rearrange("p (g c) -> p g c", g=G)
        mvd3 = mvd_t[:].rearrange("p (g c) -> p g c", g=G)
        mvab_d3 = mvab_d[:].rearrange("p (g s c) -> p (g s) c", g=G, s=2)
        mvc_d3 = mvc_d[:].rearrange("p (g c) -> p g c", g=G)
        mvd_d3 = mvd_d[:].rearrange("p (g c) -> p g c", g=G)
        # batched in blocks of 4 chunks: per-transfer bytes stay above the
        # 500ns descriptor floor while chunk 0 is ready early
        GB = 4
        mvab_b = mvab_t[:].rearrange("p (b c) -> p b c", b=G // GB)
        mvc_b = mvc_t[:].rearrange("p (b c) -> p b c", b=G // GB)
        mvd_b = mvd_t[:].rearrange("p (b c) -> p b c", b=G // GB)
        mvab_db = mvab_d[:].rearrange("p (b c) -> p b c", b=G // GB)
        mvc_db = mvc_d[:].rearrange("p (b c) -> p b c", b=G // GB)
        mvd_db = mvd_d[:].rearrange("p (b c) -> p b c", b=G // GB)
        for b in range(G // GB):
            nc.sync.dma_start(mvab_b[:, b, :], mvab_db[:, b, :])
            nc.sync.dma_start(mvc_b[:, b, :], mvc_db[:, b, :])
            nc.sync.dma_start(mvd_b[:, b, :], mvd_db[:, b, :])
        sv_t = load(sv_d, nc.sync)

        # Pool: xblk blocks 4:6 (its own operands), then bias-init tensors
        nc.gpsimd.dma_start(xblk3[:, 4:6, :], xblk_d3[:, 4:6, :])
        b2abz_t = load(b2abz_d, nc.gpsimd)
        dinit_t = load(dinit_d, nc.gpsimd)
        b2dx_t = load(b2dx_d, nc.gpsimd)
        b2c_t = load(b2c_d, nc.gpsimd)

        # warm the ACT function table (Relu) before the MLP needs it
        warm_t = work.tile([1, 1], F32)
        nc.scalar.activation(warm_t[:], nc.const_aps.tensor(0.0, (1, 1)),
                             mybir.ActivationFunctionType.Relu)

        # --- MLP: h1 = relu(W0.T wT + b0); h2 = relu(W1.T h1 + b1) ---
        wT = mlppack_t[0:16, 0:512]
        w0 = mlppack_t[0:16, 512:576]
        w1 = mlppack_t[0:64, 576:640]
        b0c = mlppack_t[0:64, 640:641]
        b1c = mlppack_t[0:64, 641:642]

        ps_h1 = ps_mlp.tile([64, E], F32, tag="mlp")
        nc.tensor.matmul(ps_h1[:], w0, wT, start=True, stop=True)
        h1_t = work.tile([64, E], BF16)
        nc.scalar.activation(h1_t[:], ps_h1[:],
                             mybir.ActivationFunctionType.Relu,
                             bias=b0c, scale=1.0)

        # --- PSUM accumulator banks: one 2KB bank per e-tile ---
        # cols 0:192 = A|B, 192:320 = D (incl. out-bias), 320+64i = C_i
        # Each bank is opened by one full-bank-width start=True matmul (the
        # A|B bias init padded with zeros); everything after accumulates
        # with start=False, ordered by WAW deps through the opener.
        banks = [ps_acc.tile([128, 512], F32, tag=f"bk{t}", name=f"bank{t}")
                 for t in range(ET)]

        # first NBC chunks: h-pair broadcast fused into a PE matmul against
        # W1 column-pairs repeated 64x (skips the h2 -> DMA latency chain)
        bct_pe = []
        for g in range(NBC):
            ps_bc = ps_mlp.tile([128, E], F32, tag="bc")
            nc.tensor.matmul(ps_bc[:], mlppack_t[0:64, 646 + 128 * g:
                                                  646 + 128 * (g + 1)],
                             h1_t[:], start=True, stop=True)
            bct = bct_pool.tile([128, E], BF16, tag="bct")
            nc.scalar.activation(bct[:], ps_bc[:],
                                 mybir.ActivationFunctionType.Relu,
                                 bias=mlppack_t[:, 642 + g:643 + g],
                                 scale=1.0)
            bct_pe.append(bct)

        ps_h2 = ps_mlp.tile([64, E], F32, tag="mlp")
        nc.tensor.matmul(ps_h2[:], w1, h1_t[:], start=True, stop=True)
        h2_t = work.tile([64, E], BF16)
        nc.scalar.activation(h2_t[:], ps_h2[:],
                             mybir.ActivationFunctionType.Relu,
                             bias=b1c, scale=1.0)

        def ets(t):
            return bass.ts(t, 128)

        # bank openers + b2 bias contributions.  The D path (and its bias
        # init, including the final out0 bias via the 1/s2 row) carries a
        # host-side 1/s2 per-sample factor and accumulates straight into
        # the A region, so out0 is just (A+D') * s2 at the end.
        for t in range(ET):
            nc.tensor.matmul(banks[t][:, 0:384], xblk3[:, 0, ets(t)],
                             b2abz_t[:, 0:384], start=True, stop=False,
                             skip_group_check=True)
            nc.tensor.matmul(banks[t][:, 0:128], dinit_t[:, ets(t)],
                             b2dx_t[:], start=False, stop=False,
                             skip_group_check=True)
            for ci, blk in enumerate((3, 4, 5)):
                nc.tensor.matmul(banks[t][:, 192 + 64 * ci:256 + 64 * ci],
                                 xblk3[0:64, blk, ets(t)], b2c_t[:],
                                 start=False, stop=False,
                                 skip_group_check=True)

        # --- main loop over 32 chunks ---
        for g in range(G):
            if g < NBC:
                bct = bct_pe[g]
            else:
                # h-pair broadcast via DMA (issued on ACT queue): rows 0:64
                # = h2[2g], rows 64:128 = h2[2g+1]
                bct = bct_pool.tile([128, E], BF16, tag="bct")
                h_src = h2_t[2 * g:2 * g + 2, :]
                h_bc = dataclasses.replace(
                    h_src, ap=[h_src.ap[0], [0, 64], [1, E]])
                nc.scalar.dma_start(bct[:], h_bc)

            # KR products: pt[:, j, :] = xblk[j] * bct
            # DVE: blocks 0:3 + first e-tile of block 3 (C0); Pool: rest.
            pt = pt_pool.tile([128, NBLK * E], BF16, tag="pt")
            pt3 = pt[:].rearrange("p (b e) -> p b e", b=NBLK)
            bct3b = dataclasses.replace(
                bct[:], ap=[bct[:].ap[0], [0, 3], [1, E]])
            bct2 = dataclasses.replace(
                bct[:], ap=[bct[:].ap[0], [0, 2], [1, E]])
            if g < 2:
                # finer grain at startup so PE can begin on block 0 early
                bct1 = dataclasses.replace(
                    bct[:], ap=[bct[:].ap[0], [0, 1], [1, E]])
                nc.vector.tensor_mul(pt3[:, 0:1, :], xblk3[:, 0:1, :], bct1)
                bct2b = dataclasses.replace(
                    bct[:], ap=[bct[:].ap[0], [0, 2], [1, E]])
                nc.vector.tensor_mul(pt3[:, 1:3, :], xblk3[:, 1:3, :], bct2b)
            else:
                nc.vector.tensor_mul(pt3[:, 0:3, :], xblk3[:, 0:3, :], bct3b)
            nc.vector.tensor_mul(pt3[:, 3, 0:256], xblk3[:, 3, 0:256],
                                 bct[:, 0:256])
            nc.gpsimd.tensor_mul(pt3[:, 3, 256:E], xblk3[:, 3, 256:E],
                                 bct[:, 256:E])
            nc.gpsimd.tensor_mul(pt3[:, 4:6, :], xblk3[:, 4:6, :], bct2)

            last = g == G - 1
            for t in range(ET):
                sl = ets(t)
                nc.tensor.matmul(banks[t][:, 0:192], pt3[:, 0, sl],
                                 mvab3[:, 2 * g, :], start=False, stop=False,
                                 skip_group_check=True)
                nc.tensor.matmul(banks[t][:, 0:192], pt3[:, 1, sl],
                                 mvab3[:, 2 * g + 1, :], start=False,
                                 stop=last, skip_group_check=True)
                nc.tensor.matmul(banks[t][:, 0:128], pt3[:, 2, sl],
                                 mvd3[:, g, :], start=False, stop=last,
                                 skip_group_check=True)
                for ci, blk in enumerate((3, 4, 5)):
                    nc.tensor.matmul(
                        banks[t][:, 192 + 64 * ci:256 + 64 * ci],
                        pt3[:, blk, sl], mvc3[:, g, :], start=False,
                        stop=last, skip_group_check=True)

        # --- epilogue: out0 = (psA + psD') * s2 on ACT (per-partition
        # scale); for out1, DVE evacuates the B|C psum region to SBUF and
        # Pool (fast, but barred from PSUM) finishes
        # out1_i = sbB*v2_i + sbC_i.
        outS = work.tile([128, ET * 320], F32)
        oS = outS[:].rearrange("p (t c) -> p t c", t=ET)
        bcS = work.tile([128, ET * 256], F32)
        bc3 = bcS[:].rearrange("p (t c) -> p t c", t=ET)
        tmp1 = work.tile([128, ET * 192], F32)
        tm3 = tmp1[:].rearrange("p (t c) -> p t c", t=ET)
        for t in range(ET):
            nc.scalar.mul(oS[:, t, 0:128], banks[t][:, 0:128],
                          sv_t[:, t:t + 1])
            nc.vector.tensor_copy(bc3[:, t, :], banks[t][:, 128:384])
            # Pool may only use plain TensorTensor: one broadcast multiply
            # covering all 3 vector components, then one add of the C part.
            sbB = bc3[:, t, 0:64]
            sbB_bc = dataclasses.replace(
                sbB, ap=[sbB.ap[0], [0, 3], [1, 64]])
            svv = sv_t[:, 4 + t:5 + t]
            svv_bc = dataclasses.replace(
                svv, ap=[svv.ap[0], [4, 3], [0, 64]])
            t3 = tm3[:, t, :].rearrange("p (i w) -> p i w", i=3)
            nc.gpsimd.tensor_mul(t3, sbB_bc, svv_bc)
            o1 = oS[:, t, 128:320].rearrange("p (w i) -> p i w", i=3)
            nc.gpsimd.tensor_add(o1, t3, bc3[:, t, 64:256].rearrange(
                "p (i w) -> p i w", i=3))
            if t == ET - 1:
                # split on the out0/out1 boundary: the halves become ready
                # at different times (ACT mul vs the copy->Pool chain)
                nc.scalar.dma_start(outp_d[ets(t), 0:128], oS[:, t, 0:128])
                nc.sync.dma_start(outp_d[ets(t), 128:320], oS[:, t, 128:320])
            else:
                eng = nc.scalar if t % 2 == 0 else nc.sync
                eng.dma_start(outp_d[ets(t), :], oS[:, t, :])

    nc.compile()
    return nc


_NC = None


def _get_nc():
    global _NC
    if _NC is None:
        _NC = _build_nc()
    return _NC


def _prep_inputs(data_in1, data_in2, weight, W0, b0, W1, b1, W2, b2, bias):
    f32 = np.float32
    data_in1 = np.ascontiguousarray(data_in1, dtype=f32)
    data_in2 = np.ascontiguousarray(data_in2, dtype=f32)
    weight = np.ascontiguousarray(weight, dtype=f32)
    W0 = np.asarray(W0, f32); b0 = np.asarray(b0, f32)
    W1 = np.asarray(W1, f32); b1 = np.asarray(b1, f32)
    W2 = np.asarray(W2, f32); b2 = np.asarray(b2, f32)
    bias = np.asarray(bias, f32)

    s1 = data_in1[:, :MUL0]                      # [N,128]
    v1 = data_in1[:, MUL0:].reshape(N, MUL1, 3)  # [N,64,3]
    s2 = data_in2[:, 0]                          # [N]
    v2 = data_in2[:, 1:4]                        # [N,3]

    def bf(x):
        return np.ascontiguousarray(x, dtype=f32).astype(BF16_NP)

    s1t = s1.T                                   # [128,N]
    vs = [(v1[:, :, i] * s2[:, None]).T for i in range(3)]   # [64,N] each
    dot12 = np.einsum("eui,ei->eu", v1, v2).T    # [64,N]

    # xblk blocks: 0=s1t, 1=[s1hi;s1lo], 2=[d2/s2;d2/s2], 3=[vs0;vs0],
    # 4=[vs1;vs1], 5=[vs2;vs2].  Block 2 carries 1/s2 so the D path can
    # share the A accumulator (out0 = (A + D') * s2 exactly).
    d2s = dot12 / s2[None, :]
    xblk_full = np.stack([
        s1t,
        np.concatenate([s1t[64:128], s1t[0:64]], axis=0),
        np.concatenate([d2s, d2s], axis=0),
        np.concatenate([vs[0], vs[0]], axis=0),
        np.concatenate([vs[1], vs[1]], axis=0),
        np.concatenate([vs[2], vs[2]], axis=0),
    ], axis=1)                                   # [128, 6, N] f32
    xblk_full = bf(xblk_full)

    # W2 instruction blocks
    Wa3 = W2[:, :N1].reshape(64, 128, 128)
    Wb3 = W2[:, N1:N1 + N2].reshape(64, 128, 64)
    Wc3 = W2[:, N1 + N2:N1 + N2 + N3].reshape(64, 64, 64)
    Wd3 = W2[:, N1 + N2 + N3:].reshape(64, 64, 128)

    # mvab[r, g, s, :]: r<64 -> k=2g, u = r (s=0) or 64+r (s=1)
    #                   r>=64 -> k=2g+1, u = r (s=0) or r-64 (s=1)
    ks = np.arange(G * 2).reshape(G, 2)          # k for (g, half)
    mvab = np.empty((128, G, 2, 192), dtype=f32)
    r_lo = np.arange(64)
    # s=0: (k=2g, u=r_lo) | (k=2g+1, u=64+r_lo)
    mvab[0:64, :, 0, 0:128] = Wa3[ks[:, 0]][:, r_lo].transpose(1, 0, 2)
    mvab[64:128, :, 0, 0:128] = Wa3[ks[:, 1]][:, 64 + r_lo].transpose(1, 0, 2)
    mvab[0:64, :, 0, 128:192] = Wb3[ks[:, 0]][:, r_lo].transpose(1, 0, 2)
    mvab[64:128, :, 0, 128:192] = Wb3[ks[:, 1]][:, 64 + r_lo].transpose(1, 0, 2)
    # s=1: (k=2g, u=64+r_lo) | (k=2g+1, u=r_lo)
    mvab[0:64, :, 1, 0:128] = Wa3[ks[:, 0]][:, 64 + r_lo].transpose(1, 0, 2)
    mvab[64:128, :, 1, 0:128] = Wa3[ks[:, 1]][:, r_lo].transpose(1, 0, 2)
    mvab[0:64, :, 1, 128:192] = Wb3[ks[:, 0]][:, 64 + r_lo].transpose(1, 0, 2)
    mvab[64:128, :, 1, 128:192] = Wb3[ks[:, 1]][:, r_lo].transpose(1, 0, 2)

    # mvc/mvd[r, g, :]: r<64 -> (k=2g, u=r); r>=64 -> (k=2g+1, u=r-64)
    mvc = np.empty((128, G, 64), dtype=f32)
    mvc[0:64] = Wc3[ks[:, 0]][:, r_lo].transpose(1, 0, 2)
    mvc[64:128] = Wc3[ks[:, 1]][:, r_lo].transpose(1, 0, 2)
    mvd = np.empty((128, G, 128), dtype=f32)
    mvd[0:64] = I3 * Wd3[ks[:, 0]][:, r_lo].transpose(1, 0, 2)
    mvd[64:128] = I3 * Wd3[ks[:, 1]][:, r_lo].transpose(1, 0, 2)

    b2abz = np.zeros((128, 512), dtype=f32)
    b2abz[:, 0:128] = b2[:N1].reshape(128, 128)
    b2abz[:, 128:192] = b2[N1:N1 + N2].reshape(128, 64)
    b2c = b2[N1 + N2:N1 + N2 + N3].reshape(64, 64)
    b2dx = np.empty((65, 128), dtype=f32)
    b2dx[0:64] = I3 * b2[N1 + N2 + N3:].reshape(64, 128)
    b2dx[64] = bias

    shared = {
        "mvab": bf(mvab.reshape(128, -1)),
        "mvc": bf(mvc.reshape(128, -1)),
        "mvd": bf(mvd.reshape(128, -1)),
        "b2abz": bf(b2abz),
        "b2c": bf(b2c),
        "b2dx": bf(b2dx),
    }

    in_maps = []
    for c in range(N_CORES):
        e0 = c * E
        m = dict(shared)
        mlppack = np.zeros((128, MLPC), dtype=f32)
        mlppack[0:16, 0:512] = weight[e0:e0 + E].T
        mlppack[0:16, 512:576] = W0
        mlppack[0:64, 576:640] = W1
        mlppack[0:64, 640] = b0
        mlppack[0:64, 641] = b1
        for g in range(NBC):
            mlppack[0:64, 642 + g] = b1[2 * g]
            mlppack[64:128, 642 + g] = b1[2 * g + 1]
            # W1 column-pair repeated 64x: col r of block g is
            # W1[:, 2g + (r >= 64)]
            mlppack[0:64, 646 + 128 * g:646 + 128 * g + 64] = \
                W1[:, 2 * g:2 * g + 1]
            mlppack[0:64, 646 + 128 * g + 64:646 + 128 * (g + 1)] = \
                W1[:, 2 * g + 1:2 * g + 2]
        m["mlppack"] = bf(mlppack)
        m["xblk"] = np.ascontiguousarray(
            xblk_full[:, :, e0:e0 + E]).reshape(128, NBLK * E)
        sv = np.empty((128, 16), dtype=f32)
        for t in range(ET):
            t0 = e0 + t * 128
            sv[:, t] = s2[t0:t0 + 128]
            for i in range(3):
                sv[:, 4 + 4 * i + t] = v2[t0:t0 + 128, i]
        m["sv"] = sv
        dinit = np.empty((65, E), dtype=f32)
        dinit[0:64] = d2s[:, e0:e0 + E]
        dinit[64] = 1.0 / s2[e0:e0 + E]
        m["dinit"] = bf(dinit)
        in_maps.append(m)
    return in_maps


def run(in_maps, **kwargs):
    nc = _get_nc()
    return run_bass_kernel_spmd(nc, in_maps, list(range(N_CORES)), **kwargs)


def kernel(data_in1, data_in2, weight, W0, b0, W1, b1, W2, b2, bias):
    in_maps = _prep_inputs(
        data_in1, data_in2, weight, W0, b0, W1, b1, W2, b2, bias
    )
    res = run(in_maps)
    out = np.concatenate(
        [np.asarray(res.results[c]["outp"]) for c in range(N_CORES)], axis=0
    )
    return out.astype(np.float32)
